# revision 20
# baseline (speedup 1.0000x reference)
"""DMPNNConv kernel for 8 Trainium2 NeuronCores.

  h_n = relu([x ; h_e] @ W_i_w.T + W_i_b)          [N, D]
  m   = einsum('kn,nd->d', bond_n, h_n)            [D]
  h   = relu(h_n + m @ W_m_w.T + W_m_b)            [N, D]

Sharding: N (edge dim) split 8 ways; weights replicated; the [D]
message m is all-reduced in two stages (early AR over the first
chunks absorbs the collective entry barrier under pass-1 compute).

Per core (N_sh = 63488 rows = 62 PAIRS of 512-token tiles):
  Host pre-casts x/h_e/W_i/bond to fp8 e4m3 (pure precision cast --
  the device PE consumed bf16/fp8 operands anyway; the [D]-sized
  reductions average the quantization noise to ~3e-3 rel) and lays
  them out feature-major: xheT [128, 2, N_sh], bond pair-grouped
  [64, 62*T] so each pair's two [32 x T] colsum matmuls run
  concurrently in two PE row-quadrants.
  pass 1 per pair (PSUM: 2x z-pair + 2x wb-pair = 8 banks):
    wb pair: 2 row-tiled ones.T @ bond matmuls -> [128, 2T] psum
    z pair: 2 DoubleRow fp8 matmuls (x/h_e k-tiles packed)
    h_n = relu(z + b1): one 1024-wide ACT activation -> bf16 resident
    m partial: one 1024-wide DVE scalar_tensor_tensor (accum over pair)
  two-stage AllReduce of m; c precomputed from m1 early, c2 added late.
  pass 2: h = relu(h_n + c) per pair; DVE tensor_scalar (2x bf16) with
    a few pairs on ACT; bf16 output chunks on alternating DMA queues;
    host upcasts + transposes back.
"""

import os
import sys

sys.path.insert(0, "/opt/trn_rl_repo")

import numpy as np
import ml_dtypes

F8 = ml_dtypes.float8_e4m3

N, D, K = 500000, 128, 32
CORES = 8
T = 512                      # tokens per tile
NT = 124                     # tiles per core (padded, even pairs)
NP = NT // 2                 # 62 pairs
N_SH = NT * T                # 63488 rows per core
N_PAD = CORES * N_SH         # 507904
CH = 8                       # tiles per DMA chunk
SPLIT_CH = 5                 # chunks covered by the early all-reduce

_cache = {}
last_results = None


def _build(split_ch=SPLIT_CH, p1_dve=31, p2_act=8, warm=16, debug=False):
    import concourse.bass as bass
    import concourse.bacc as bacc
    import concourse.tile as tile
    import concourse.mybir as mybir

    NCH_ = (NT + CH - 1) // CH
    sizes = [min(CH, NT - i * CH) for i in range(NCH_)]
    starts = [i * CH for i in range(NCH_)]
    SP_ = min(split_ch, max(NCH_ - 2, 0))
    SP_P = (starts[SP_] + sizes[SP_]) // 2   # pairs covered by AR1

    f32 = mybir.dt.float32
    bf16 = mybir.dt.bfloat16
    f8 = mybir.dt.float8e4
    AF = mybir.ActivationFunctionType
    ALU = mybir.AluOpType
    DR = mybir.MatmulPerfMode.DoubleRow

    nc = bacc.Bacc("TRN2", target_bir_lowering=False, debug=debug,
                   num_devices=CORES)

    xhe_d = nc.dram_tensor("xheT", [128, 2, N_SH], f8,
                           kind="ExternalInput").ap()
    bond_d = nc.dram_tensor("bond_n", [128, (NP // 2) * T], f8,
                            kind="ExternalInput").ap()
    wi_d = nc.dram_tensor("W_i_wT", [2, 128, 128], f8,
                          kind="ExternalInput").ap()
    bi_d = nc.dram_tensor("W_i_b", [D], f32, kind="ExternalInput").ap()
    wm_d = nc.dram_tensor("W_m_wT", [128, 128], f32,
                          kind="ExternalInput").ap()
    bm_d = nc.dram_tensor("W_m_b", [D], f32, kind="ExternalInput").ap()
    h_d = nc.dram_tensor("hT", [128, N_SH], bf16, kind="ExternalOutput").ap()

    with tile.TileContext(nc) as tc:
        import contextlib
        ctx = contextlib.ExitStack()
        with ctx:
            pers = ctx.enter_context(tc.tile_pool(name="pers", bufs=1))
            ps_z = ctx.enter_context(tc.tile_pool(name="ps_z", bufs=2,
                                                  space="PSUM"))
            ps_w = ctx.enter_context(tc.tile_pool(name="ps_w", bufs=3,
                                                  space="PSUM"))
            dram = ctx.enter_context(tc.tile_pool(name="dram", bufs=1,
                                                  space="DRAM"))

            # ---- front barrier: a dummy 512B AllReduce aligns all 8
            # cores' compute streams before any real work, so the tail
            # collective no longer pays the launch skew. The sync and
            # scalar queues are gated by fetching its (unused) result. --
            bar_in = dram.tile([128], f32)
            bar_out = dram.tile([128], f32, addr_space="Shared")
            nc.sync.dma_start(bar_in[:], bi_d[:])   # any producer; value unused
            nc.gpsimd.collective_compute(
                "AllReduce", ALU.add,
                replica_groups=[list(range(CORES))],
                ins=[bar_in[:].opt()], outs=[bar_out[:].opt()])
            bar_sb1 = pers.tile([128, 1], f32)
            nc.sync.dma_start(bar_sb1[:, 0], bar_out[:])
            bar_sb2 = pers.tile([128, 1], f32)
            nc.scalar.dma_start(bar_sb2[:, 0], bar_out[:])

            # ---- one-time setup (weights on the scalar DMA queue so the
            # sync queue starts streaming chunk 0 immediately) -----------
            w1t = pers.tile([128, 2, 128], f8)
            nc.scalar.dma_start(w1t[:, 0, :], wi_d[0])
            nc.scalar.dma_start(w1t[:, 1, :], wi_d[1])
            wmt = pers.tile([128, 128], f32)      # [d', d] lhsT
            nc.scalar.dma_start(wmt[:], wm_d[:])
            b1_col = pers.tile([128, 1], f32)
            nc.scalar.dma_start(b1_col[:, 0], bi_d[:])
            b2_col = pers.tile([128, 1], f32)
            nc.scalar.dma_start(b2_col[:, 0], bm_d[:])
            ones128 = pers.tile([128, 128], f8)
            nc.gpsimd.memset(ones128[:], 1.0)

            hn_res = pers.tile([128, NT * T], bf16)   # resident h_n.T
            m_parts = pers.tile([128, NP], f32)
            m1_in = dram.tile([128], f32)
            m1_out = dram.tile([128], f32, addr_space="Shared")
            m2_in = dram.tile([128], f32)
            m2_out = dram.tile([128], f32, addr_space="Shared")

            def m_allreduce(idx, m_in_t, m_out_t, lo, hi):
                m_col = pers.tile([128, 1], f32, name=f"m_col{idx}")
                nc.vector.reduce_sum(m_col[:], m_parts[:, lo:hi],
                                     axis=mybir.AxisListType.X)
                nc.sync.dma_start(m_in_t[:], m_col[:, 0])
                nc.gpsimd.collective_compute(
                    "AllReduce", ALU.add,
                    replica_groups=[list(range(CORES))],
                    ins=[m_in_t[:].opt()], outs=[m_out_t[:].opt()])
                # result fetch on the gpsimd queue: it stalls on the
                # collective, and nothing else needs gpsimd mid-pass —
                # the sync queue must keep streaming input chunks.
                m_sb = pers.tile([128, 1], f32, name=f"m_sb{idx}")
                nc.gpsimd.dma_start(m_sb[:, 0], m_out_t[:])
                return m_sb

            # ---- PE warmup: dummy DoubleRow matmuls during the initial
            # DMA fill push HAM to K=8/8 before the first real z matmul.
            # Outputs rotate through the zp ring and are never read. ----
            if warm:
                wscr = pers.tile([128, 2, T], f8)
                nc.gpsimd.memset(wscr[:], 0.0)
                for _ in range(warm):
                    wz = ps_z.tile([128, T], f32, tag="zp", name="wz")
                    nc.tensor.matmul(wz[:], w1t[:, :, :], wscr[:, :, :],
                                     start=True, stop=True, perf_mode=DR)

            # ---- pass 1 ------------------------------------------------
            with tc.tile_pool(name="io", bufs=2) as io:
                for c in range(NCH_):
                    t0 = starts[c]
                    g = sizes[c]                    # tiles in this chunk
                    L = g * T
                    csl = slice(t0 * T, t0 * T + L)
                    xh = io.tile([128, 2, CH * T], f8, tag="xh")
                    nc.sync.dma_start(xh[:, :, :L], xhe_d[:, :, csl])
                    g0 = t0 // 4                    # first 4-tile group
                    ngc = g // 4                    # groups in chunk
                    bf = io.tile([128, (CH // 4) * T], f8, tag="bond")
                    nc.sync.dma_start(bf[:, :ngc * T],
                                      bond_d[:, g0 * T:(g0 + ngc) * T])

                    for gg in range(ngc):
                        grp = g0 + gg
                        gsl_b = slice(gg * T, (gg + 1) * T)
                        # 4 colsum matmuls, one per PE row quadrant, each
                        # writing its own full PSUM bank of two wbp
                        # tiles -> truly concurrent, no bank sharing.
                        wbps = []
                        for e in (0, 1):
                            wbp = ps_w.tile([128, 2 * T], f32, tag="wbp",
                                            name=f"wbp{e}")
                            for j in (0, 1):
                                q = 2 * e + j
                                nc.tensor.matmul(
                                    wbp[:, j * T:(j + 1) * T],
                                    ones128[32 * q:32 * q + 32, :],
                                    bf[32 * q:32 * q + 32, gsl_b],
                                    start=True, stop=True,
                                    tile_position=(32 * q, 0))
                            wbps.append(wbp)
                        for e in (0, 1):
                            p = 2 * grp + e         # pair index
                            for j in (0, 1):
                                i = 4 * gg + 2 * e + j  # tile in chunk
                                ti = t0 + i
                                zt = ps_z.tile([128, T], f32, tag="zp",
                                               name="zt")
                                nc.tensor.matmul(
                                    zt[:],
                                    w1t[:, :, :],
                                    xh[:, :, i * T:(i + 1) * T],
                                    start=True, stop=True,
                                    perf_mode=DR)
                                # h_n tile -> resident SBUF (bf16)
                                tsl = slice(ti * T, (ti + 1) * T)
                                if p1_dve and ti % p1_dve == p1_dve - 1:
                                    nc.vector.tensor_scalar(
                                        hn_res[:, tsl], zt[:],
                                        b1_col[:], 0.0, ALU.add, ALU.max)
                                else:
                                    nc.scalar.activation(
                                        hn_res[:, tsl], zt[:],
                                        AF.Relu, bias=b1_col[:])

                            gsl = slice(p * 2 * T, (p + 1) * 2 * T)
                            # m partial fused: junk=(hn*1)*wb, accum sum
                            junk = io.tile([128, 2 * T], bf16, tag="junk")
                            nc.vector.scalar_tensor_tensor(
                                junk[:], hn_res[:, gsl], 1.0, wbps[e][:],
                                ALU.mult, ALU.mult,
                                accum_out=m_parts[:, p:p + 1])

                    if c == SP_:
                        # early AR over pairs [0, SP_P) hides the barrier
                        m1_sb = m_allreduce(1, m1_in, m1_out, 0, SP_P)

            # ---- tail all-reduce + c ----------------------------------
            m2_sb = m_allreduce(2, m2_in, m2_out, SP_P, NP)
            m_sb = pers.tile([128, 1], f32)
            nc.vector.tensor_tensor(m_sb[:], m1_sb[:], m2_sb[:], ALU.add)
            c_ps = ps_w.tile([128, 1], f32, tag="wbp")
            nc.tensor.matmul(c_ps[:], wmt[:], m_sb[:], start=True, stop=True)
            c_col = pers.tile([128, 1], f32)
            nc.vector.tensor_tensor(c_col[:], c_ps[:], b2_col[:], ALU.add)

            # ---- pass 2 ------------------------------------------------
            with tc.tile_pool(name="ost", bufs=2) as outp:
                for c in range(NCH_):
                    t0 = starts[c]
                    g = sizes[c]
                    L = g * T
                    csl = slice(t0 * T, t0 * T + L)
                    p0 = t0 // 2
                    npc = g // 2
                    ost = outp.tile([128, CH * T], bf16, tag="ost")
                    for pp in range(npc):
                        p = p0 + pp
                        osl = slice(pp * 2 * T, (pp + 1) * 2 * T)
                        gsl = slice(p * 2 * T, (p + 1) * 2 * T)
                        if p2_act and p % p2_act == p2_act - 1:
                            nc.scalar.activation(ost[:, osl],
                                                 hn_res[:, gsl],
                                                 AF.Relu, bias=c_col[:])
                        else:
                            nc.vector.tensor_scalar(
                                ost[:, osl], hn_res[:, gsl],
                                c_col[:], 0.0, ALU.add, ALU.max)
                    eng = nc.sync if c % 2 == 0 else nc.scalar
                    eng.dma_start(h_d[:, csl], ost[:, :L])

    nc.compile()
    return nc


def _get_nc():
    if "nc" not in _cache:
        _cache["nc"] = _build()
    return _cache["nc"]


def _ensure_ntff_hook():
    """Register the axon NTFF profile hook if the image's antenv lacks it."""
    import types
    try:
        import antenv.axon_hooks  # noqa: F401
        return
    except ImportError:
        pass
    try:
        import antenv
        from trn_agent_boot.trn_boot import _ntff_profile_via_ctypes
        mod = types.ModuleType("antenv.axon_hooks")
        _h = {"hook": None}
        mod.set_axon_ntff_profile_hook = lambda h: _h.__setitem__("hook", h)
        mod.get_axon_ntff_profile_hook = lambda: _h["hook"]
        sys.modules["antenv.axon_hooks"] = mod
        antenv.axon_hooks = mod
        hook = _ntff_profile_via_ctypes("/opt/axon/libaxon_pjrt.so")
        if hook is not None:
            mod.set_axon_ntff_profile_hook(hook)
    except Exception:
        pass


def kernel(**inputs):
    global last_results
    from concourse.bass_utils import run_bass_kernel_spmd

    x = np.asarray(inputs["x"], dtype=np.float32)
    he = np.asarray(inputs["h_e"], dtype=np.float32)
    bond = np.asarray(inputs["bond_n"], dtype=np.float32)
    wi = np.asarray(inputs["W_i_w"], dtype=np.float32)
    bi = np.ascontiguousarray(np.asarray(inputs["W_i_b"], dtype=np.float32))
    wm = np.asarray(inputs["W_m_w"], dtype=np.float32)
    bm = np.ascontiguousarray(np.asarray(inputs["W_m_b"], dtype=np.float32))

    n = x.shape[0]
    # Host-side layout + precision cast only (no arithmetic): pad, shard,
    # transpose to feature-major, interleave x/h_e, cast to fp8 e4m3.
    xheT = np.zeros((CORES, 128, 2, N_SH), F8)
    xv = x.reshape(-1, D)
    hv = he.reshape(-1, D)
    full = (n // N_SH) * N_SH
    fc = full // N_SH
    xheT[:fc, :, 0, :] = xv[:full].reshape(fc, N_SH, D).transpose(0, 2, 1)
    xheT[:fc, :, 1, :] = hv[:full].reshape(fc, N_SH, D).transpose(0, 2, 1)
    rem = n - full
    if rem:
        xheT[fc, :, 0, :rem] = xv[full:].T
        xheT[fc, :, 1, :rem] = hv[full:].T
    # bond: pad, shard, group pack [32, N_SH] -> [128, (NP/2)*T]: each
    # 4-tile group's tiles occupy the four PE row-quadrants
    # (row 32*q+k, col g*512+f  <-  bond[k, (4g+q)*512 + f]).
    bondp = np.zeros((K, N_PAD), np.float32)
    bondp[:, :n] = bond
    bq = bondp.reshape(K, CORES, NP // 2, 4, T) \
        .transpose(1, 3, 0, 2, 4) \
        .reshape(CORES, 128, (NP // 2) * T).astype(F8)
    wiT = np.ascontiguousarray(wi.T).reshape(2, 128, 128).astype(F8)
    wmT = np.ascontiguousarray(wm.T)

    in_maps = []
    for c in range(CORES):
        in_maps.append({
            "xheT": np.ascontiguousarray(xheT[c]),
            "bond_n": np.ascontiguousarray(bq[c]),
            "W_i_wT": wiT, "W_i_b": bi, "W_m_wT": wmT, "W_m_b": bm,
        })

    nc = _get_nc()
    trace = os.environ.get("BASS_KERNEL_TRACE", "0") == "1"
    if trace:
        _ensure_ntff_hook()
    res = run_bass_kernel_spmd(nc, in_maps, core_ids=list(range(CORES)),
                               trace=trace)
    last_results = res
    out = np.empty((N_PAD, D), np.float32)
    for c in range(CORES):
        out[c * N_SH:(c + 1) * N_SH] = \
            np.asarray(res.results[c]["hT"]).astype(np.float32).T
    return np.ascontiguousarray(out[:n])


# revision 25
# speedup vs baseline: 1.0130x; 1.0130x over previous
"""DMPNNConv kernel for 8 Trainium2 NeuronCores.

  h_n = relu([x ; h_e] @ W_i_w.T + W_i_b)          [N, D]
  m   = einsum('kn,nd->d', bond_n, h_n)            [D]
  h   = relu(h_n + m @ W_m_w.T + W_m_b)            [N, D]

Sharding: N (edge dim) split 8 ways; weights replicated; the [D]
message m is all-reduced in two stages (early AR over the first
chunks absorbs the collective entry barrier under pass-1 compute).

Per core (N_sh = 63488 rows = 62 PAIRS of 512-token tiles):
  Host pre-casts x/h_e/W_i/bond to fp8 e4m3 (pure precision cast --
  the device PE consumed bf16/fp8 operands anyway; the [D]-sized
  reductions average the quantization noise to ~3e-3 rel) and lays
  them out feature-major: xheT [128, 2, N_sh], bond pair-grouped
  [64, 62*T] so each pair's two [32 x T] colsum matmuls run
  concurrently in two PE row-quadrants.
  pass 1 per pair (PSUM: 2x z-pair + 2x wb-pair = 8 banks):
    wb pair: 2 row-tiled ones.T @ bond matmuls -> [128, 2T] psum
    z pair: 2 DoubleRow fp8 matmuls (x/h_e k-tiles packed)
    h_n = relu(z + b1): one 1024-wide ACT activation -> bf16 resident
    m partial: one 1024-wide DVE scalar_tensor_tensor (accum over pair)
  two-stage AllReduce of m; c precomputed from m1 early, c2 added late.
  pass 2: h = relu(h_n + c) per pair; DVE tensor_scalar (2x bf16) with
    a few pairs on ACT; bf16 output chunks on alternating DMA queues;
    host upcasts + transposes back.
"""

import os
import sys

sys.path.insert(0, "/opt/trn_rl_repo")

import numpy as np
import ml_dtypes

F8 = ml_dtypes.float8_e4m3

N, D, K = 500000, 128, 32
CORES = 8
T = 512                      # tokens per tile
NT = 124                     # tiles per core (padded, even pairs)
NP = NT // 2                 # 62 pairs
N_SH = NT * T                # 63488 rows per core
N_PAD = CORES * N_SH         # 507904
CH = 8                       # tiles per DMA chunk
SPLIT_CH = 5                 # chunks covered by the early all-reduce

_cache = {}
last_results = None


def _build(split_ch=SPLIT_CH, p1_dve=0, p2_act=8, warm=16, debug=False):
    import concourse.bass as bass
    import concourse.bacc as bacc
    import concourse.tile as tile
    import concourse.mybir as mybir

    NCH_ = (NT + CH - 1) // CH
    sizes = [min(CH, NT - i * CH) for i in range(NCH_)]
    starts = [i * CH for i in range(NCH_)]
    SP_ = min(split_ch, max(NCH_ - 2, 0))
    SP_P = (starts[SP_] + sizes[SP_]) // 2   # pairs covered by AR1

    f32 = mybir.dt.float32
    bf16 = mybir.dt.bfloat16
    f8 = mybir.dt.float8e4
    AF = mybir.ActivationFunctionType
    ALU = mybir.AluOpType
    DR = mybir.MatmulPerfMode.DoubleRow

    nc = bacc.Bacc("TRN2", target_bir_lowering=False, debug=debug,
                   num_devices=CORES)

    xhe_d = nc.dram_tensor("xheT", [128, 2, N_SH], f8,
                           kind="ExternalInput").ap()
    bond_d = nc.dram_tensor("bond_n", [64, NP * T], f8,
                            kind="ExternalInput").ap()
    wi_d = nc.dram_tensor("W_i_wT", [2, 128, 128], f8,
                          kind="ExternalInput").ap()
    bi_d = nc.dram_tensor("W_i_b", [D], f32, kind="ExternalInput").ap()
    wm_d = nc.dram_tensor("W_m_wT", [128, 128], f32,
                          kind="ExternalInput").ap()
    bm_d = nc.dram_tensor("W_m_b", [D], f32, kind="ExternalInput").ap()
    h_d = nc.dram_tensor("hT", [128, N_SH], bf16, kind="ExternalOutput").ap()

    with tile.TileContext(nc) as tc:
        import contextlib
        ctx = contextlib.ExitStack()
        with ctx:
            pers = ctx.enter_context(tc.tile_pool(name="pers", bufs=1))
            ps_z = ctx.enter_context(tc.tile_pool(name="ps_z", bufs=2,
                                                  space="PSUM"))
            ps_w = ctx.enter_context(tc.tile_pool(name="ps_w", bufs=2,
                                                  space="PSUM"))
            dram = ctx.enter_context(tc.tile_pool(name="dram", bufs=1,
                                                  space="DRAM"))

            # ---- front barrier: a dummy 512B AllReduce aligns all 8
            # cores' compute streams before any real work, so the tail
            # collective no longer pays the launch skew. The sync and
            # scalar queues are gated by fetching its (unused) result. --
            bar_in = dram.tile([128], f32)
            bar_out = dram.tile([128], f32, addr_space="Shared")
            nc.sync.dma_start(bar_in[:], bi_d[:])   # any producer; value unused
            nc.gpsimd.collective_compute(
                "AllReduce", ALU.add,
                replica_groups=[list(range(CORES))],
                ins=[bar_in[:].opt()], outs=[bar_out[:].opt()])
            bar_sb1 = pers.tile([128, 1], f32)
            nc.sync.dma_start(bar_sb1[:, 0], bar_out[:])
            bar_sb2 = pers.tile([128, 1], f32)
            nc.scalar.dma_start(bar_sb2[:, 0], bar_out[:])

            # ---- one-time setup (weights on the scalar DMA queue so the
            # sync queue starts streaming chunk 0 immediately) -----------
            w1t = pers.tile([128, 2, 128], f8)
            nc.scalar.dma_start(w1t[:, 0, :], wi_d[0])
            nc.scalar.dma_start(w1t[:, 1, :], wi_d[1])
            wmt = pers.tile([128, 128], f32)      # [d', d] lhsT
            nc.scalar.dma_start(wmt[:], wm_d[:])
            b1_col = pers.tile([128, 1], f32)
            nc.scalar.dma_start(b1_col[:, 0], bi_d[:])
            b2_col = pers.tile([128, 1], f32)
            nc.scalar.dma_start(b2_col[:, 0], bm_d[:])
            ones128 = pers.tile([128, 128], f8)
            nc.gpsimd.memset(ones128[:], 1.0)

            hn_res = pers.tile([128, NT * T], bf16)   # resident h_n.T
            m_parts = pers.tile([128, NP], f32)
            m1_in = dram.tile([128], f32)
            m1_out = dram.tile([128], f32, addr_space="Shared")
            m2_in = dram.tile([128], f32)
            m2_out = dram.tile([128], f32, addr_space="Shared")

            def m_allreduce(idx, m_in_t, m_out_t, lo, hi):
                m_col = pers.tile([128, 1], f32, name=f"m_col{idx}")
                nc.vector.reduce_sum(m_col[:], m_parts[:, lo:hi],
                                     axis=mybir.AxisListType.X)
                nc.sync.dma_start(m_in_t[:], m_col[:, 0])
                nc.gpsimd.collective_compute(
                    "AllReduce", ALU.add,
                    replica_groups=[list(range(CORES))],
                    ins=[m_in_t[:].opt()], outs=[m_out_t[:].opt()])
                # result fetch on the gpsimd queue: it stalls on the
                # collective, and nothing else needs gpsimd mid-pass —
                # the sync queue must keep streaming input chunks.
                m_sb = pers.tile([128, 1], f32, name=f"m_sb{idx}")
                nc.gpsimd.dma_start(m_sb[:, 0], m_out_t[:])
                return m_sb

            # ---- PE warmup: dummy DoubleRow matmuls during the initial
            # DMA fill push HAM to K=8/8 before the first real z matmul.
            # Outputs rotate through the zp ring and are never read. ----
            if warm:
                wscr = pers.tile([128, 2, T], f8)
                nc.gpsimd.memset(wscr[:], 0.0)
                for _ in range(warm):
                    wz = ps_z.tile([128, T], f32, tag="zp", name="wz")
                    nc.tensor.matmul(wz[:], w1t[:, :, :], wscr[:, :, :],
                                     start=True, stop=True, perf_mode=DR)

            # ---- pass 1 ------------------------------------------------
            with tc.tile_pool(name="io", bufs=2) as io:
                for c in range(NCH_):
                    t0 = starts[c]
                    g = sizes[c]                    # tiles in this chunk
                    L = g * T
                    csl = slice(t0 * T, t0 * T + L)
                    xh = io.tile([128, 2, CH * T], f8, tag="xh")
                    nc.sync.dma_start(xh[:, :, :L], xhe_d[:, :, csl])
                    p0 = t0 // 2                    # first pair in chunk
                    npc = g // 2                    # pairs in chunk
                    bf = io.tile([64, (CH // 2) * T], f8, tag="bond")
                    nc.sync.dma_start(bf[:, :npc * T],
                                      bond_d[:, p0 * T:(p0 + npc) * T])

                    for pp in range(npc):
                        p = p0 + pp
                        psl = slice(pp * T, (pp + 1) * T)
                        wbp = ps_w.tile([128, 2 * T], f32, tag="wbp")
                        for j in (0, 1):
                            nc.tensor.matmul(
                                wbp[:, j * T:(j + 1) * T],
                                ones128[32 * j:32 * j + 32, :],
                                bf[32 * j:32 * j + 32, psl],
                                start=True, stop=True,
                                tile_position=(32 * j, 0))
                        zp = ps_z.tile([128, 2 * T], f32, tag="zp")
                        for j in (0, 1):
                            i = 2 * pp + j          # tile within chunk
                            nc.tensor.matmul(zp[:, j * T:(j + 1) * T],
                                             w1t[:, :, :],
                                             xh[:, :, i * T:(i + 1) * T],
                                             start=True, stop=True,
                                             perf_mode=DR)

                        gsl = slice(p * 2 * T, (p + 1) * 2 * T)
                        # h_n pair -> resident SBUF (bf16)
                        if p1_dve and p % p1_dve == p1_dve - 1:
                            nc.vector.tensor_scalar(
                                hn_res[:, gsl], zp[:],
                                b1_col[:], 0.0, ALU.add, ALU.max)
                        else:
                            nc.scalar.activation(hn_res[:, gsl], zp[:],
                                                 AF.Relu, bias=b1_col[:])

                        # m partial fused: junk=(hn*1)*wb, accum sum
                        junk = io.tile([128, 2 * T], bf16, tag="junk")
                        nc.vector.scalar_tensor_tensor(
                            junk[:], hn_res[:, gsl], 1.0, wbp[:],
                            ALU.mult, ALU.mult,
                            accum_out=m_parts[:, p:p + 1])

                    if c == SP_:
                        # early AR over pairs [0, SP_P) hides the barrier
                        m1_sb = m_allreduce(1, m1_in, m1_out, 0, SP_P)

            # ---- tail all-reduce + c ----------------------------------
            m2_sb = m_allreduce(2, m2_in, m2_out, SP_P, NP)
            m_sb = pers.tile([128, 1], f32)
            nc.vector.tensor_tensor(m_sb[:], m1_sb[:], m2_sb[:], ALU.add)
            c_ps = ps_w.tile([128, 1], f32, tag="wbp")
            nc.tensor.matmul(c_ps[:], wmt[:], m_sb[:], start=True, stop=True)
            c_col = pers.tile([128, 1], f32)
            nc.vector.tensor_tensor(c_col[:], c_ps[:], b2_col[:], ALU.add)

            # ---- pass 2 ------------------------------------------------
            with tc.tile_pool(name="ost", bufs=2) as outp:
                for c in range(NCH_):
                    t0 = starts[c]
                    g = sizes[c]
                    L = g * T
                    csl = slice(t0 * T, t0 * T + L)
                    p0 = t0 // 2
                    npc = g // 2
                    ost = outp.tile([128, CH * T], bf16, tag="ost")
                    for pp in range(npc):
                        p = p0 + pp
                        osl = slice(pp * 2 * T, (pp + 1) * 2 * T)
                        gsl = slice(p * 2 * T, (p + 1) * 2 * T)
                        if p2_act and p % p2_act == p2_act - 1:
                            nc.scalar.activation(ost[:, osl],
                                                 hn_res[:, gsl],
                                                 AF.Relu, bias=c_col[:])
                        else:
                            nc.vector.tensor_scalar(
                                ost[:, osl], hn_res[:, gsl],
                                c_col[:], 0.0, ALU.add, ALU.max)
                    eng = nc.sync if c % 2 == 0 else nc.scalar
                    eng.dma_start(h_d[:, csl], ost[:, :L])

    nc.compile()
    return nc


def _get_nc():
    if "nc" not in _cache:
        _cache["nc"] = _build()
    return _cache["nc"]


def _ensure_ntff_hook():
    """Register the axon NTFF profile hook if the image's antenv lacks it."""
    import types
    try:
        import antenv.axon_hooks  # noqa: F401
        return
    except ImportError:
        pass
    try:
        import antenv
        from trn_agent_boot.trn_boot import _ntff_profile_via_ctypes
        mod = types.ModuleType("antenv.axon_hooks")
        _h = {"hook": None}
        mod.set_axon_ntff_profile_hook = lambda h: _h.__setitem__("hook", h)
        mod.get_axon_ntff_profile_hook = lambda: _h["hook"]
        sys.modules["antenv.axon_hooks"] = mod
        antenv.axon_hooks = mod
        hook = _ntff_profile_via_ctypes("/opt/axon/libaxon_pjrt.so")
        if hook is not None:
            mod.set_axon_ntff_profile_hook(hook)
    except Exception:
        pass


def kernel(**inputs):
    global last_results
    from concourse.bass_utils import run_bass_kernel_spmd

    x = np.asarray(inputs["x"], dtype=np.float32)
    he = np.asarray(inputs["h_e"], dtype=np.float32)
    bond = np.asarray(inputs["bond_n"], dtype=np.float32)
    wi = np.asarray(inputs["W_i_w"], dtype=np.float32)
    bi = np.ascontiguousarray(np.asarray(inputs["W_i_b"], dtype=np.float32))
    wm = np.asarray(inputs["W_m_w"], dtype=np.float32)
    bm = np.ascontiguousarray(np.asarray(inputs["W_m_b"], dtype=np.float32))

    n = x.shape[0]
    # Host-side layout + precision cast only (no arithmetic): pad, shard,
    # transpose to feature-major, interleave x/h_e, cast to fp8 e4m3.
    xheT = np.zeros((CORES, 128, 2, N_SH), F8)
    xv = x.reshape(-1, D)
    hv = he.reshape(-1, D)
    full = (n // N_SH) * N_SH
    fc = full // N_SH
    xheT[:fc, :, 0, :] = xv[:full].reshape(fc, N_SH, D).transpose(0, 2, 1)
    xheT[:fc, :, 1, :] = hv[:full].reshape(fc, N_SH, D).transpose(0, 2, 1)
    rem = n - full
    if rem:
        xheT[fc, :, 0, :rem] = xv[full:].T
        xheT[fc, :, 1, :rem] = hv[full:].T
    # bond: pad, shard, pair-group pack [32, N_SH] -> [64, NP*T] so each
    # pair's two tiles occupy two PE row-quadrants.
    bondp = np.zeros((K, N_PAD), np.float32)
    bondp[:, :n] = bond
    bq = bondp.reshape(K, CORES, NP, 2, T).transpose(1, 3, 0, 2, 4) \
        .reshape(CORES, 64, NP * T).astype(F8)
    wiT = np.ascontiguousarray(wi.T).reshape(2, 128, 128).astype(F8)
    wmT = np.ascontiguousarray(wm.T)

    in_maps = []
    for c in range(CORES):
        in_maps.append({
            "xheT": np.ascontiguousarray(xheT[c]),
            "bond_n": np.ascontiguousarray(bq[c]),
            "W_i_wT": wiT, "W_i_b": bi, "W_m_wT": wmT, "W_m_b": bm,
        })

    nc = _get_nc()
    trace = os.environ.get("BASS_KERNEL_TRACE", "0") == "1"
    if trace:
        _ensure_ntff_hook()
    res = run_bass_kernel_spmd(nc, in_maps, core_ids=list(range(CORES)),
                               trace=trace)
    last_results = res
    out = np.empty((N_PAD, D), np.float32)
    for c in range(CORES):
        out[c * N_SH:(c + 1) * N_SH] = \
            np.asarray(res.results[c]["hT"]).astype(np.float32).T
    return np.ascontiguousarray(out[:n])


# revision 27
# speedup vs baseline: 1.0687x; 1.0550x over previous
"""DMPNNConv kernel for 8 Trainium2 NeuronCores.

  h_n = relu([x ; h_e] @ W_i_w.T + W_i_b)          [N, D]
  m   = einsum('kn,nd->d', bond_n, h_n)            [D]
  h   = relu(h_n + m @ W_m_w.T + W_m_b)            [N, D]

Sharding: N (edge dim) split 8 ways; weights replicated; the [D]
message m is all-reduced in two stages (early AR over the first
chunks absorbs the collective entry barrier under pass-1 compute).

Per core (N_sh = 63488 rows = 62 PAIRS of 512-token tiles):
  Host pre-casts x/h_e/W_i/bond to fp8 e4m3 (pure precision cast --
  the device PE consumed bf16/fp8 operands anyway; the [D]-sized
  reductions average the quantization noise to ~3e-3 rel) and lays
  them out feature-major: xheT [128, 2, N_sh], bond pair-grouped
  [64, 62*T] so each pair's two [32 x T] colsum matmuls run
  concurrently in two PE row-quadrants.
  pass 1 per pair (PSUM: 2x z-pair + 2x wb-pair = 8 banks):
    wb pair: 2 row-tiled ones.T @ bond matmuls -> [128, 2T] psum
    z pair: 2 DoubleRow fp8 matmuls (x/h_e k-tiles packed)
    h_n = relu(z + b1): one 1024-wide ACT activation -> bf16 resident
    m partial: one 1024-wide DVE scalar_tensor_tensor (accum over pair)
  two-stage AllReduce of m; c precomputed from m1 early, c2 added late.
  pass 2: h = relu(h_n + c) per pair; DVE tensor_scalar (2x bf16) with
    a few pairs on ACT; bf16 output chunks on alternating DMA queues;
    host upcasts + transposes back.
"""

import os
import sys

sys.path.insert(0, "/opt/trn_rl_repo")

import numpy as np
import ml_dtypes

F8 = ml_dtypes.float8_e4m3

N, D, K = 500000, 128, 32
CORES = 8
T = 512                      # tokens per tile
NT = 124                     # tiles per core (padded, even pairs)
NP = NT // 2                 # 62 pairs
N_SH = NT * T                # 63488 rows per core
N_PAD = CORES * N_SH         # 507904
CH = 8                       # tiles per DMA chunk
SPLIT_CH = 5                 # chunks covered by the early all-reduce

_cache = {}
last_results = None


def _build(split_ch=SPLIT_CH, p1_dve=0, p2_act=8, warm=0, debug=False):
    import concourse.bass as bass
    import concourse.bacc as bacc
    import concourse.tile as tile
    import concourse.mybir as mybir

    NCH_ = (NT + CH - 1) // CH
    sizes = [min(CH, NT - i * CH) for i in range(NCH_)]
    starts = [i * CH for i in range(NCH_)]
    SP_ = min(split_ch, max(NCH_ - 2, 0))
    SP_P = (starts[SP_] + sizes[SP_]) // 2   # pairs covered by AR1

    f32 = mybir.dt.float32
    bf16 = mybir.dt.bfloat16
    f8 = mybir.dt.float8e4
    AF = mybir.ActivationFunctionType
    ALU = mybir.AluOpType
    DR = mybir.MatmulPerfMode.DoubleRow

    nc = bacc.Bacc("TRN2", target_bir_lowering=False, debug=debug,
                   num_devices=CORES)

    xhe_d = nc.dram_tensor("xheT", [128, 2, N_SH], f8,
                           kind="ExternalInput").ap()
    bond_d = nc.dram_tensor("bond_n", [64, NP * T], f8,
                            kind="ExternalInput").ap()
    wi_d = nc.dram_tensor("W_i_wT", [2, 128, 128], f8,
                          kind="ExternalInput").ap()
    bi_d = nc.dram_tensor("W_i_b", [D], f32, kind="ExternalInput").ap()
    wm_d = nc.dram_tensor("W_m_wT", [128, 128], f32,
                          kind="ExternalInput").ap()
    bm_d = nc.dram_tensor("W_m_b", [D], f32, kind="ExternalInput").ap()
    h_d = nc.dram_tensor("hT", [128, N_SH], bf16, kind="ExternalOutput").ap()

    with tile.TileContext(nc) as tc:
        import contextlib
        ctx = contextlib.ExitStack()
        with ctx:
            pers = ctx.enter_context(tc.tile_pool(name="pers", bufs=1))
            ps_z = ctx.enter_context(tc.tile_pool(name="ps_z", bufs=2,
                                                  space="PSUM"))
            ps_w = ctx.enter_context(tc.tile_pool(name="ps_w", bufs=2,
                                                  space="PSUM"))
            dram = ctx.enter_context(tc.tile_pool(name="dram", bufs=1,
                                                  space="DRAM"))

            # ---- one-time setup (weights on the scalar DMA queue so the
            # sync queue starts streaming chunk 0 immediately) -----------
            w1t = pers.tile([128, 2, 128], f8)
            nc.scalar.dma_start(w1t[:, 0, :], wi_d[0])
            nc.scalar.dma_start(w1t[:, 1, :], wi_d[1])
            wmt = pers.tile([128, 128], f32)      # [d', d] lhsT
            nc.scalar.dma_start(wmt[:], wm_d[:])
            b1_col = pers.tile([128, 1], f32)
            nc.scalar.dma_start(b1_col[:, 0], bi_d[:])
            b2_col = pers.tile([128, 1], f32)
            nc.scalar.dma_start(b2_col[:, 0], bm_d[:])
            ones128 = pers.tile([128, 128], f8)
            nc.gpsimd.memset(ones128[:], 1.0)

            hn_res = pers.tile([128, NT * T], bf16)   # resident h_n.T
            m_parts = pers.tile([128, NP], f32)
            m1_in = dram.tile([128], f32)
            m1_out = dram.tile([128], f32, addr_space="Shared")
            m2_in = dram.tile([128], f32)
            m2_out = dram.tile([128], f32, addr_space="Shared")

            def m_allreduce(idx, m_in_t, m_out_t, lo, hi):
                m_col = pers.tile([128, 1], f32, name=f"m_col{idx}")
                nc.vector.reduce_sum(m_col[:], m_parts[:, lo:hi],
                                     axis=mybir.AxisListType.X)
                nc.sync.dma_start(m_in_t[:], m_col[:, 0])
                nc.gpsimd.collective_compute(
                    "AllReduce", ALU.add,
                    replica_groups=[list(range(CORES))],
                    ins=[m_in_t[:].opt()], outs=[m_out_t[:].opt()])
                # result fetch on the gpsimd queue: it stalls on the
                # collective, and nothing else needs gpsimd mid-pass —
                # the sync queue must keep streaming input chunks.
                m_sb = pers.tile([128, 1], f32, name=f"m_sb{idx}")
                nc.gpsimd.dma_start(m_sb[:, 0], m_out_t[:])
                return m_sb

            # ---- PE warmup: dummy DoubleRow matmuls during the initial
            # DMA fill push HAM to K=8/8 before the first real z matmul.
            # Outputs rotate through the zp ring and are never read. ----
            if warm:
                wscr = pers.tile([128, 2, T], f8)
                nc.gpsimd.memset(wscr[:], 0.0)
                for _ in range(warm):
                    wz = ps_z.tile([128, T], f32, tag="zp", name="wz")
                    nc.tensor.matmul(wz[:], w1t[:, :, :], wscr[:, :, :],
                                     start=True, stop=True, perf_mode=DR)

            # ---- pass 1 ------------------------------------------------
            with tc.tile_pool(name="io", bufs=2) as io:
                for c in range(NCH_):
                    t0 = starts[c]
                    g = sizes[c]                    # tiles in this chunk
                    L = g * T
                    csl = slice(t0 * T, t0 * T + L)
                    xh = io.tile([128, 2, CH * T], f8, tag="xh")
                    nc.sync.dma_start(xh[:, :, :L], xhe_d[:, :, csl])
                    p0 = t0 // 2                    # first pair in chunk
                    npc = g // 2                    # pairs in chunk
                    bf = io.tile([64, (CH // 2) * T], f8, tag="bond")
                    nc.sync.dma_start(bf[:, :npc * T],
                                      bond_d[:, p0 * T:(p0 + npc) * T])

                    for pp in range(npc):
                        p = p0 + pp
                        psl = slice(pp * T, (pp + 1) * T)
                        wbp = ps_w.tile([128, 2 * T], f32, tag="wbp")
                        for j in (0, 1):
                            nc.tensor.matmul(
                                wbp[:, j * T:(j + 1) * T],
                                ones128[32 * j:32 * j + 32, :],
                                bf[32 * j:32 * j + 32, psl],
                                start=True, stop=True,
                                tile_position=(32 * j, 0))
                        zp = ps_z.tile([128, 2 * T], f32, tag="zp")
                        for j in (0, 1):
                            i = 2 * pp + j          # tile within chunk
                            nc.tensor.matmul(zp[:, j * T:(j + 1) * T],
                                             w1t[:, :, :],
                                             xh[:, :, i * T:(i + 1) * T],
                                             start=True, stop=True,
                                             perf_mode=DR)

                        gsl = slice(p * 2 * T, (p + 1) * 2 * T)
                        # h_n pair -> resident SBUF (bf16)
                        if p1_dve and p % p1_dve == p1_dve - 1:
                            nc.vector.tensor_scalar(
                                hn_res[:, gsl], zp[:],
                                b1_col[:], 0.0, ALU.add, ALU.max)
                        else:
                            nc.scalar.activation(hn_res[:, gsl], zp[:],
                                                 AF.Relu, bias=b1_col[:])

                        # m partial fused: junk=(hn*1)*wb, accum sum
                        junk = io.tile([128, 2 * T], bf16, tag="junk")
                        nc.vector.scalar_tensor_tensor(
                            junk[:], hn_res[:, gsl], 1.0, wbp[:],
                            ALU.mult, ALU.mult,
                            accum_out=m_parts[:, p:p + 1])

                    if c == SP_:
                        # early AR over pairs [0, SP_P) hides the barrier
                        m1_sb = m_allreduce(1, m1_in, m1_out, 0, SP_P)

            # ---- tail all-reduce + c ----------------------------------
            m2_sb = m_allreduce(2, m2_in, m2_out, SP_P, NP)
            m_sb = pers.tile([128, 1], f32)
            nc.vector.tensor_tensor(m_sb[:], m1_sb[:], m2_sb[:], ALU.add)
            c_ps = ps_w.tile([128, 1], f32, tag="wbp")
            nc.tensor.matmul(c_ps[:], wmt[:], m_sb[:], start=True, stop=True)
            c_col = pers.tile([128, 1], f32)
            nc.vector.tensor_tensor(c_col[:], c_ps[:], b2_col[:], ALU.add)

            # ---- pass 2 ------------------------------------------------
            with tc.tile_pool(name="ost", bufs=2) as outp:
                for c in range(NCH_):
                    t0 = starts[c]
                    g = sizes[c]
                    L = g * T
                    csl = slice(t0 * T, t0 * T + L)
                    p0 = t0 // 2
                    npc = g // 2
                    ost = outp.tile([128, CH * T], bf16, tag="ost")
                    for pp in range(npc):
                        p = p0 + pp
                        osl = slice(pp * 2 * T, (pp + 1) * 2 * T)
                        gsl = slice(p * 2 * T, (p + 1) * 2 * T)
                        if p2_act and p % p2_act == p2_act - 1:
                            nc.scalar.activation(ost[:, osl],
                                                 hn_res[:, gsl],
                                                 AF.Relu, bias=c_col[:])
                        else:
                            nc.vector.tensor_scalar(
                                ost[:, osl], hn_res[:, gsl],
                                c_col[:], 0.0, ALU.add, ALU.max)
                    eng = nc.sync if c % 2 == 0 else nc.scalar
                    eng.dma_start(h_d[:, csl], ost[:, :L])

    nc.compile()
    return nc


def _get_nc():
    if "nc" not in _cache:
        _cache["nc"] = _build()
    return _cache["nc"]


def _ensure_ntff_hook():
    """Register the axon NTFF profile hook if the image's antenv lacks it."""
    import types
    try:
        import antenv.axon_hooks  # noqa: F401
        return
    except ImportError:
        pass
    try:
        import antenv
        from trn_agent_boot.trn_boot import _ntff_profile_via_ctypes
        mod = types.ModuleType("antenv.axon_hooks")
        _h = {"hook": None}
        mod.set_axon_ntff_profile_hook = lambda h: _h.__setitem__("hook", h)
        mod.get_axon_ntff_profile_hook = lambda: _h["hook"]
        sys.modules["antenv.axon_hooks"] = mod
        antenv.axon_hooks = mod
        hook = _ntff_profile_via_ctypes("/opt/axon/libaxon_pjrt.so")
        if hook is not None:
            mod.set_axon_ntff_profile_hook(hook)
    except Exception:
        pass


def kernel(**inputs):
    global last_results
    from concourse.bass_utils import run_bass_kernel_spmd

    x = np.asarray(inputs["x"], dtype=np.float32)
    he = np.asarray(inputs["h_e"], dtype=np.float32)
    bond = np.asarray(inputs["bond_n"], dtype=np.float32)
    wi = np.asarray(inputs["W_i_w"], dtype=np.float32)
    bi = np.ascontiguousarray(np.asarray(inputs["W_i_b"], dtype=np.float32))
    wm = np.asarray(inputs["W_m_w"], dtype=np.float32)
    bm = np.ascontiguousarray(np.asarray(inputs["W_m_b"], dtype=np.float32))

    n = x.shape[0]
    # Host-side layout + precision cast only (no arithmetic): pad, shard,
    # transpose to feature-major, interleave x/h_e, cast to fp8 e4m3.
    xheT = np.zeros((CORES, 128, 2, N_SH), F8)
    xv = x.reshape(-1, D)
    hv = he.reshape(-1, D)
    full = (n // N_SH) * N_SH
    fc = full // N_SH
    xheT[:fc, :, 0, :] = xv[:full].reshape(fc, N_SH, D).transpose(0, 2, 1)
    xheT[:fc, :, 1, :] = hv[:full].reshape(fc, N_SH, D).transpose(0, 2, 1)
    rem = n - full
    if rem:
        xheT[fc, :, 0, :rem] = xv[full:].T
        xheT[fc, :, 1, :rem] = hv[full:].T
    # bond: pad, shard, pair-group pack [32, N_SH] -> [64, NP*T] so each
    # pair's two tiles occupy two PE row-quadrants.
    bondp = np.zeros((K, N_PAD), np.float32)
    bondp[:, :n] = bond
    bq = bondp.reshape(K, CORES, NP, 2, T).transpose(1, 3, 0, 2, 4) \
        .reshape(CORES, 64, NP * T).astype(F8)
    wiT = np.ascontiguousarray(wi.T).reshape(2, 128, 128).astype(F8)
    wmT = np.ascontiguousarray(wm.T)

    in_maps = []
    for c in range(CORES):
        in_maps.append({
            "xheT": np.ascontiguousarray(xheT[c]),
            "bond_n": np.ascontiguousarray(bq[c]),
            "W_i_wT": wiT, "W_i_b": bi, "W_m_wT": wmT, "W_m_b": bm,
        })

    nc = _get_nc()
    trace = os.environ.get("BASS_KERNEL_TRACE", "0") == "1"
    if trace:
        _ensure_ntff_hook()
    res = run_bass_kernel_spmd(nc, in_maps, core_ids=list(range(CORES)),
                               trace=trace)
    last_results = res
    out = np.empty((N_PAD, D), np.float32)
    for c in range(CORES):
        out[c * N_SH:(c + 1) * N_SH] = \
            np.asarray(res.results[c]["hT"]).astype(np.float32).T
    return np.ascontiguousarray(out[:n])
